# revision 23
# baseline (speedup 1.0000x reference)
"""Multi-headed causal attention (B=2, S=2048, D=1024, H=16, DK=DV=64) on 8
Trainium2 NeuronCores — v2.

Sharding: 2 groups of 4 cores, one group per batch element. Within a group,
core j owns two 256-query stripes (blocks j and 7-j, balanced causal work)
and computes the K/V projections only for its own 512-key slice; the slices
are AllGathered (bf16, 2 collectives) inside the group so every core sees the
full projected K^T [1024, 2048] and V [2048, 1024+ones]. This removes the 4x
K/V-projection duplication of the zero-communication scheme.

All matmul inputs are bf16 (1 cycle/row at any free size, FWL weight loads,
half DMA). Scores for the two heads of a pair sit in PE partitions 0:64 /
64:128, so the pair's score matmuls run concurrently in disjoint row groups.
Softmax: no max-subtraction (scores are O(1)); exp runs on ScalarE over wide
[128,1024] PSUM tiles; causal+padding masking is one bf16 multiplicative
mask per tile (host data, uniform program). The denominator comes from a
64-wide ones block in the AV stationary operand (rows 64:128 of the AV PSUM
are 64 copies of the softmax denominator), the reciprocal is one approx-recip
on a single row, PE-replicated back into rows 64:128 of the same bank, and
one VectorE multiply normalizes + casts the per-pair attention output for the
PSUM-accumulated bf16 output projection.
"""

import numpy as np

B, S, D, H, DK = 2, 2048, 1024, 16, 64
NQ = 512          # queries per core: 2 stripes x 256
KSL = 512         # keys projected per core
NCORES = 8
NPAIR = 8         # head pairs
NKT = 16          # 128-key tiles

_BUILT = {}

# exp/mask tile groups: 4 "lo" groups of 2 key-tiles (free 512, both stripes)
# + 2 "hi" groups of 4 key-tiles (free 256, stripe B only)
GROUPS = [(0, (0, 1)), (1, (2, 3)), (2, (4, 5)), (3, (6, 7)),
          (4, (8, 9, 10, 11)), (5, (12, 13, 14, 15))]


def _build_nc():
    import os
    PH = int(os.environ.get("BISECT_PHASES", "9"))
    import concourse.bacc as bacc
    import concourse.mybir as mybir
    from concourse import tile
    from concourse.dve_ops import (
        RECIP_APPROX_FAST_CONSTS as RECIP_CONSTS,
        RECIPROCAL_APPROX_FAST as RECIP_FAST,
    )

    f32 = mybir.dt.float32
    f32r = mybir.dt.float32r
    bf16 = mybir.dt.bfloat16
    AF = mybir.ActivationFunctionType
    ALU = mybir.AluOpType

    nc = bacc.Bacc("TRN2", target_bir_lowering=False, debug=False,
                   num_devices=NCORES)

    wk_t = nc.declare_dram_parameter("wk_t", [D, D], bf16, isOutput=False)
    wv_t = nc.declare_dram_parameter("wv_t", [D, D], bf16, isOutput=False)
    wq_t = nc.declare_dram_parameter("wq_t", [D, D], bf16, isOutput=False)
    wo_t = nc.declare_dram_parameter("wo_t", [D, D], bf16, isOutput=False)
    xk_sl = nc.declare_dram_parameter("xk_sl", [D, KSL], bf16, isOutput=False)
    xv_sl = nc.declare_dram_parameter("xv_sl", [D, KSL], bf16, isOutput=False)
    xk_lo = nc.declare_dram_parameter("xk_lo", [D, KSL], bf16, isOutput=False)
    xv_lo = nc.declare_dram_parameter("xv_lo", [D, KSL], bf16, isOutput=False)
    xq_sl = nc.declare_dram_parameter("xq_sl", [D, NQ], bf16, isOutput=False)
    bk_s = nc.declare_dram_parameter("bk_s", [128, 8], f32, isOutput=False)
    bq_s = nc.declare_dram_parameter("bq_s", [128, 8], f32, isOutput=False)
    bv_r = nc.declare_dram_parameter("bv_r", [1, D], f32r, isOutput=False)
    bo_r = nc.declare_dram_parameter("bo_r", [1, D], f32r, isOutput=False)
    ones1 = nc.declare_dram_parameter("ones1", [1, 128], f32r, isOutput=False)
    onesb = nc.declare_dram_parameter("onesb", [128, D], bf16, isOutput=False)
    msk_lo = nc.declare_dram_parameter("msk_lo", [128, 8 * NQ], bf16,
                                       isOutput=False)
    msk_hi = nc.declare_dram_parameter("msk_hi", [128, 8 * 256], bf16,
                                       isOutput=False)
    out = nc.declare_dram_parameter("out", [NQ, D], f32, isOutput=True)
    DBG = int(os.environ.get("DEBUG_DUMPS", "0"))
    if DBG:
        dbg_kt = nc.declare_dram_parameter("dbg_kt", [128, S], bf16,
                                           isOutput=True)
        dbg_v = nc.declare_dram_parameter("dbg_v", [128, 2048], bf16,
                                          isOutput=True)
        dbg_q = nc.declare_dram_parameter("dbg_q", [128, NQ], bf16,
                                          isOutput=True)
        dbg_nav = nc.declare_dram_parameter("dbg_nav", [128, NQ], bf16,
                                            isOutput=True)
        dbg_av = nc.declare_dram_parameter("dbg_av", [128, NQ], f32,
                                           isOutput=True)

    RG = [[0, 1, 2, 3], [4, 5, 6, 7]]

    from contextlib import ExitStack

    class _Stop(Exception):
        pass

    with tile.TileContext(nc) as tc:
      try:
        with ExitStack() as ctx:
            persist = ctx.enter_context(tc.tile_pool(name="persist", bufs=1))
            dram = ctx.enter_context(
                tc.tile_pool(name="dram", bufs=1, space="DRAM"))

            # ---- collective warmup: a tiny AllGather issued first absorbs
            # the ~90us per-execution CC-path setup concurrently with the
            # projection phase ----
            dag_i = dram.tile([4, 512], bf16, name="dagi")
            dag_o = dram.tile([16, 512], bf16, name="dago")
            nc.gpsimd.collective_compute(
                "AllGather", mybir.AluOpType.bypass, replica_groups=RG,
                ins=[dag_i[:].opt()], outs=[dag_o[:].opt()])

            # ---- constants ----
            bk_sb = persist.tile([128, 8], f32, name="bk")
            bq_sb = persist.tile([128, 8], f32, name="bq")
            ones_sb = persist.tile([1, 128], f32r, name="ones1")
            nc.sync.dma_start(bk_sb[:], bk_s[:])
            nc.sync.dma_start(bq_sb[:], bq_s[:])
            nc.sync.dma_start(ones_sb[:], ones1[:])
            msk_lo_sb = persist.tile([128, 8 * NQ], bf16, name="msklo")
            msk_hi_sb = persist.tile([128, 8 * 256], bf16, name="mskhi")
            nc.sync.dma_start(msk_lo_sb[:], msk_lo[:])
            nc.sync.dma_start(msk_hi_sb[:], msk_hi[:])

            # replicate bv across partitions (K=1 matmul)
            bv_rep = persist.tile([128, D], f32, name="bvrep")
            with tc.tile_pool(name="ps0", bufs=2, space="PSUM") as ps0, \
                 tc.tile_pool(name="p0s", bufs=1) as p0s:
                bv_rsb = p0s.tile([1, D], f32r, name="bvr")
                nc.sync.dma_start(bv_rsb[:], bv_r[:])
                for half in range(2):
                    rp = ps0.tile([128, 512], f32, name="rep0", tag="rep0")
                    nc.tensor.matmul(rp[:], ones_sb[:],
                                     bv_rsb[:, half * 512:(half + 1) * 512],
                                     start=True, stop=True)
                    nc.scalar.copy(bv_rep[:, half * 512:(half + 1) * 512],
                                   rp[:])

            # ---- AllGather DRAM bounce tiles ----
            ag1_in = dram.tile([D, KSL], bf16, name="ag1i")
            ag1_out = dram.tile([4 * D, KSL], bf16, name="ag1o")
            ag2_in = dram.tile([KSL, D], bf16, name="ag2i")
            ag2_out = dram.tile([4 * KSL, D], bf16, name="ag2o")

            # ---- resident attention tensors ----
            kT_sb = [persist.tile([128, S], bf16, name=f"kt{hp}")
                     for hp in range(NPAIR)]
            v_sb = [persist.tile([128, 2048], bf16, name=f"v{kt}")
                    for kt in range(NKT)]
            qT_sb = [persist.tile([128, NQ], bf16, name=f"qt{hp}")
                     for hp in range(NPAIR)]
            navTn = [persist.tile([128, NQ], bf16, name=f"nv{hp}")
                     for hp in range(NPAIR)]

            # ---- P1: K projection slice + AllGather #1 ----
            with tc.tile_pool(name="wkx", bufs=1) as wkp, \
                 tc.tile_pool(name="ktsl", bufs=1) as ktsp, \
                 tc.tile_pool(name="psk", bufs=3, space="PSUM") as psk:
                wk_sb = [wkp.tile([128, D], bf16, name=f"wk{kp}",
                                  tag=f"w{kp}") for kp in range(8)]
                xk_sb = [wkp.tile([128, KSL], bf16, name=f"xk{kp}",
                                  tag=f"x{kp}") for kp in range(8)]
                for kp in range(8):
                    nc.sync.dma_start(wk_sb[kp][:],
                                      wk_t[kp * 128:(kp + 1) * 128, :])
                    nc.sync.dma_start(xk_sb[kp][:],
                                      xk_sl[kp * 128:(kp + 1) * 128, :])
                for ft in range(8):
                    ps = psk.tile([128, KSL], f32, name="pk", tag="pk")
                    for kp in range(8):
                        nc.tensor.matmul(
                            ps[:], wk_sb[kp][:, ft * 128:(ft + 1) * 128],
                            xk_sb[kp][:], start=(kp == 0), stop=(kp == 7))
                    kt_sl = ktsp.tile([128, KSL], bf16, name=f"ksl{ft}",
                                      tag=f"ksl{ft}")
                    nc.scalar.activation(kt_sl[:], ps[:], AF.Identity,
                                         bias=bk_sb[:, ft:ft + 1])
                    nc.sync.dma_start(ag1_in[ft * 128:(ft + 1) * 128, :],
                                      kt_sl[:])
                nc.gpsimd.collective_compute(
                    "AllGather", mybir.AluOpType.bypass, replica_groups=RG,
                    ins=[ag1_in[:].opt()], outs=[ag1_out[:].opt()])
                # local duplicate of keys 0:512 (kt 0-3): lets attention
                # groups 0-1 start before the gather lands
                xkl_sb = [wkp.tile([128, KSL], bf16, name=f"xkl{kp}",
                                   tag=f"xl{kp}") for kp in range(8)]
                for kp in range(8):
                    nc.sync.dma_start(xkl_sb[kp][:],
                                      xk_lo[kp * 128:(kp + 1) * 128, :])
                for ft in range(8):
                    ps = psk.tile([128, KSL], f32, name="pk", tag="pk")
                    for kp in range(8):
                        nc.tensor.matmul(
                            ps[:], wk_sb[kp][:, ft * 128:(ft + 1) * 128],
                            xkl_sb[kp][:], start=(kp == 0), stop=(kp == 7))
                    nc.scalar.activation(kT_sb[ft][:, 0:KSL], ps[:],
                                         AF.Identity,
                                         bias=bk_sb[:, ft:ft + 1])

            if PH < 2:
                raise _Stop()
            # ---- P2: V projection slice + AllGather #2 ----
            with tc.tile_pool(name="wvx", bufs=1) as wvp, \
                 tc.tile_pool(name="vsl", bufs=2) as vslp, \
                 tc.tile_pool(name="psv", bufs=3, space="PSUM") as psv:
                wv_sb = [wvp.tile([128, D], bf16, name=f"wv{kp}",
                                  tag=f"w{kp}") for kp in range(8)]
                xv_sb = [wvp.tile([128, KSL], bf16, name=f"xv{kp}",
                                  tag=f"x{kp}") for kp in range(8)]
                for kp in range(8):
                    nc.sync.dma_start(wv_sb[kp][:],
                                      wv_t[kp * 128:(kp + 1) * 128, :])
                    nc.sync.dma_start(xv_sb[kp][:],
                                      xv_sl[kp * 128:(kp + 1) * 128, :])
                for st in range(4):
                    v_sl = vslp.tile([128, D], bf16, name="vsl", tag="vsl")
                    for half in range(2):
                        ps = psv.tile([128, 512], f32, name="pv", tag="pv")
                        for kp in range(8):
                            nc.tensor.matmul(
                                ps[:], xv_sb[kp][:, st * 128:(st + 1) * 128],
                                wv_sb[kp][:, half * 512:(half + 1) * 512],
                                start=(kp == 0), stop=(kp == 7))
                        nc.vector.tensor_tensor(
                            v_sl[:, half * 512:(half + 1) * 512], ps[:],
                            bv_rep[:, half * 512:(half + 1) * 512], ALU.add)
                    nc.sync.dma_start(ag2_in[st * 128:(st + 1) * 128, :],
                                      v_sl[:])
                nc.gpsimd.collective_compute(
                    "AllGather", mybir.AluOpType.bypass, replica_groups=RG,
                    ins=[ag2_in[:].opt()], outs=[ag2_out[:].opt()])
                # local duplicate of V for keys 0:512 straight into v_sb
                xvl_sb = [wvp.tile([128, KSL], bf16, name=f"xvl{kp}",
                                   tag=f"xl{kp}") for kp in range(8)]
                for kp in range(8):
                    nc.sync.dma_start(xvl_sb[kp][:],
                                      xv_lo[kp * 128:(kp + 1) * 128, :])
                for st in range(4):
                    for half in range(2):
                        ps = psv.tile([128, 512], f32, name="pv", tag="pv")
                        for kp in range(8):
                            nc.tensor.matmul(
                                ps[:], xvl_sb[kp][:, st * 128:(st + 1) * 128],
                                wv_sb[kp][:, half * 512:(half + 1) * 512],
                                start=(kp == 0), stop=(kp == 7))
                        nc.vector.tensor_tensor(
                            v_sb[st][:].rearrange(
                                "p (h c) -> p h c",
                                c=128)[:, half * 8:(half + 1) * 8, 64:128],
                            ps[:].rearrange("p (h c) -> p h c", c=64),
                            bv_rep[:, half * 512:(half + 1) * 512].rearrange(
                                "p (h c) -> p h c", c=64),
                            ALU.add)

            if PH < 3:
                raise _Stop()
            # ---- P3: Q projection (own 512 queries) ----
            with tc.tile_pool(name="wqx", bufs=1) as wqp, \
                 tc.tile_pool(name="psq", bufs=3, space="PSUM") as psq:
                wq_sb = [wqp.tile([128, D], bf16, name=f"wq{kp}",
                                  tag=f"w{kp}") for kp in range(8)]
                xq_sb = [wqp.tile([128, NQ], bf16, name=f"xq{kp}",
                                  tag=f"x{kp}") for kp in range(8)]
                for kp in range(8):
                    nc.sync.dma_start(wq_sb[kp][:],
                                      wq_t[kp * 128:(kp + 1) * 128, :])
                    nc.sync.dma_start(xq_sb[kp][:],
                                      xq_sl[kp * 128:(kp + 1) * 128, :])
                for ft in range(8):
                    ps = psq.tile([128, NQ], f32, name="pq", tag="pq")
                    for kp in range(8):
                        nc.tensor.matmul(
                            ps[:], wq_sb[kp][:, ft * 128:(ft + 1) * 128],
                            xq_sb[kp][:], start=(kp == 0), stop=(kp == 7))
                    nc.scalar.activation(qT_sb[ft][:], ps[:], AF.Identity,
                                         bias=bq_sb[:, ft:ft + 1])

            if PH < 4:
                raise _Stop()
            # ---- P4: load gathered kT / v into SBUF ----
            for hp in range(NPAIR):
                for c in range(1, 4):
                    nc.sync.dma_start(
                        kT_sb[hp][:, c * 512:(c + 1) * 512],
                        ag1_out[c * 1024 + hp * 128:c * 1024 + (hp + 1) * 128,
                                :])
            # v tile layout per head: [ones(64) | v(64)] — the ones block
            # puts the softmax denominator at AV-PSUM rows 0:64 (partition 0,
            # where the custom-DVE reciprocal can read it), head outputs at
            # rows 64:128.
            for kt in range(NKT):
                if kt >= 4:
                    nc.sync.dma_start(
                        v_sb[kt][:].rearrange("p (h c) -> p h c",
                                              c=128)[:, :, 64:128],
                        ag2_out[kt * 128:(kt + 1) * 128, :].rearrange(
                            "p (h c) -> p h c", c=64))
                nc.sync.dma_start(
                    v_sb[kt][:].rearrange("p (h c) -> p h c", c=128)[:, :,
                                                                    0:64],
                    onesb[:].rearrange("p (h c) -> p h c", c=64))

            if DBG:
                nc.sync.dma_start(dbg_kt[:], kT_sb[0][:])
                nc.sync.dma_start(dbg_v[:], v_sb[0][:])
                nc.sync.dma_start(dbg_q[:], qT_sb[0][:])

            # prefetch Wo weights + bias (used in P6)
            wop = ctx.enter_context(tc.tile_pool(name="wox", bufs=1))
            wo_sb = [wop.tile([128, D], bf16, name=f"wo{hp}")
                     for hp in range(NPAIR)]
            for hp in range(NPAIR):
                nc.sync.dma_start(wo_sb[hp][:],
                                  wo_t[hp * 128:(hp + 1) * 128, :])
            bo_rsb = wop.tile([1, D], f32r, name="bor")
            nc.sync.dma_start(bo_rsb[:], bo_r[:])

            if PH < 5:
                raise _Stop()
            # ---- P5: attention ----
            with tc.tile_pool(name="amp", bufs=4) as amp, \
                 tc.tile_pool(name="recp", bufs=4) as recp, \
                 tc.tile_pool(name="psc", bufs=2, space="PSUM") as psc, \
                 tc.tile_pool(name="psrep", bufs=2, space="PSUM") as psrep, \
                 tc.tile_pool(name="psav", bufs=2, space="PSUM") as psav:
                for hp in range(NPAIR):
                    av = [psav.tile([128, NQ], f32, name=f"av{i}", tag="av")
                          for i in range(2)]
                    for g, kts in GROUPS:
                        lo = g < 4
                        free = 512 if lo else 256
                        qoff = 0 if lo else 256
                        sc = [psc.tile([128, 1024], f32, name=f"sc{i}",
                                       tag="sc") for i in range(2)]
                        for ki, kt in enumerate(kts):
                            coff = ki * free
                            for i in range(2):
                                nc.tensor.matmul(
                                    sc[i][:, coff:coff + free],
                                    kT_sb[hp][64 * i:64 * i + 64,
                                              kt * 128:(kt + 1) * 128],
                                    qT_sb[hp][64 * i:64 * i + 64,
                                              qoff:qoff + free],
                                    start=True, stop=True)
                        msk = (msk_lo_sb[:, g * 1024:(g + 1) * 1024] if lo
                               else msk_hi_sb[:, (g - 4) * 1024:
                                              (g - 3) * 1024])
                        am = [amp.tile([128, 1024], bf16, name=f"am{i}",
                                       tag="am") for i in range(2)]
                        for i in range(2):
                            nc.scalar.activation(am[i][:], sc[i][:], AF.Exp,
                                                 scale=0.125)
                            eng = nc.vector if lo else nc.gpsimd
                            eng.tensor_tensor(am[i][:], am[i][:], msk,
                                              ALU.mult)
                        for ki, kt in enumerate(kts):
                            coff = ki * free
                            for i in range(2):
                                nc.tensor.matmul(
                                    av[i][:, qoff:qoff + free],
                                    v_sb[kt][:, (2 * hp + i) * 128:
                                             (2 * hp + i + 1) * 128],
                                    am[i][:, coff:coff + free],
                                    start=(kt == 0), stop=(kt == 15))
                    if DBG and hp == 0:
                        avd = amp.tile([128, NQ], f32, name="avd",
                                       tag="avd")
                        nc.vector.tensor_copy(avd[:], av[0][:])
                        nc.sync.dma_start(dbg_av[:], avd[:])
                    # normalization: rows 0:64 of av hold 64 copies of the
                    # softmax denominator (ones block in the AV stationary),
                    # rows 64:128 the head's attention output
                    for i in range(2):
                        rec = recp.tile([1, NQ], f32r, name="rec", tag="rec")
                        with nc.allow_low_precision(
                                reason="f32r recip feeds PE replicate"):
                            nc.vector._custom_dve(
                                RECIP_FAST, out=rec[:], in0=av[i][0:1, :],
                                s0=RECIP_CONSTS["s0"], s1=RECIP_CONSTS["s1"],
                                imm2=RECIP_CONSTS["imm2"])
                        rep = psrep.tile([64, NQ], f32, name="rep",
                                         tag="rep")
                        nc.tensor.matmul(rep[:], ones_sb[0:1, 0:64],
                                         rec[:], start=True, stop=True)
                        nc.vector.tensor_copy(
                            navTn[hp][64 * i:64 * i + 64, :],
                            av[i][64:128, :])
                        nc.vector.tensor_tensor(
                            navTn[hp][64 * i:64 * i + 64, :],
                            navTn[hp][64 * i:64 * i + 64, :],
                            rep[:], ALU.mult)

            if DBG:
                nc.sync.dma_start(dbg_nav[:], navTn[0][:])
            if PH < 6:
                raise _Stop()
            # ---- P6: output projection, all head pairs PSUM-accumulated ----
            with tc.tile_pool(name="fop", bufs=3) as fop, \
                 tc.tile_pool(name="bop", bufs=1) as bop, \
                 tc.tile_pool(name="pso", bufs=4, space="PSUM") as pso:
                bo_rep = bop.tile([128, D], f32, name="borep")
                for half in range(2):
                    rp = pso.tile([128, 512], f32, name="po", tag="po")
                    nc.tensor.matmul(rp[:], ones_sb[:],
                                     bo_rsb[:, half * 512:(half + 1) * 512],
                                     start=True, stop=True)
                    nc.scalar.copy(bo_rep[:, half * 512:(half + 1) * 512],
                                   rp[:])
                for oc in range(2):
                    for rc in range(4):
                        ps = pso.tile([128, 512], f32, name="po", tag="po")
                        for hp in range(NPAIR):
                            nc.tensor.matmul(
                                ps[:], navTn[hp][:, rc * 128:(rc + 1) * 128],
                                wo_sb[hp][:, oc * 512:(oc + 1) * 512],
                                start=(hp == 0), stop=(hp == 7))
                        fo = fop.tile([128, 512], f32, name="fo", tag="fo")
                        nc.vector.tensor_tensor(
                            fo[:], ps[:], bo_rep[:, oc * 512:(oc + 1) * 512],
                            ALU.add)
                        nc.sync.dma_start(
                            out[rc * 128:(rc + 1) * 128,
                                oc * 512:(oc + 1) * 512], fo[:])
      except _Stop:
          pass
    nc.compile()
    return nc


def kernel(V, K, Q, padding_mask, Wv_w, Wv_b, Wk_w, Wk_b, Wq_w, Wq_b,
           Wo_w, Wo_b):
    from concourse.bass_utils import run_bass_kernel_spmd
    import ml_dtypes

    bf = ml_dtypes.bfloat16
    V = np.asarray(V, np.float32)
    K = np.asarray(K, np.float32)
    Q = np.asarray(Q, np.float32)
    padding_mask = np.asarray(padding_mask)

    if "nc" not in _BUILT:
        _BUILT["nc"] = _build_nc()
    nc = _BUILT["nc"]

    wk_t = np.ascontiguousarray(np.asarray(Wk_w, np.float32).T).astype(bf)
    wv_t = np.ascontiguousarray(np.asarray(Wv_w, np.float32).T).astype(bf)
    wq_t = np.ascontiguousarray(np.asarray(Wq_w, np.float32).T).astype(bf)
    wo_t = np.ascontiguousarray(np.asarray(Wo_w, np.float32).T).astype(bf)
    bk_s = np.ascontiguousarray(np.asarray(Wk_b, np.float32).reshape(8, 128).T)
    bq_s = np.ascontiguousarray(np.asarray(Wq_b, np.float32).reshape(8, 128).T)
    bv_r = np.asarray(Wv_b, np.float32).reshape(1, D)
    bo_r = np.asarray(Wo_b, np.float32).reshape(1, D)
    ones1 = np.ones((1, 128), np.float32)
    onesb = np.ones((128, D), bf)

    xk_T = [np.ascontiguousarray(K[b].T).astype(bf) for b in range(B)]
    xv_T = [np.ascontiguousarray(V[b].T).astype(bf) for b in range(B)]

    in_maps = []
    blocks = []
    for core in range(NCORES):
        b, j = core // 4, core % 4
        blkA, blkB = j, 7 - j
        blocks.append((b, blkA, blkB))
        rows = np.r_[256 * blkA:256 * (blkA + 1), 256 * blkB:256 * (blkB + 1)]
        xq = np.ascontiguousarray(Q[b][rows].T).astype(bf)
        qpos = np.r_[np.arange(256 * blkA, 256 * (blkA + 1)),
                     np.arange(256 * blkB, 256 * (blkB + 1))]
        pad = (padding_mask[b] != 0)
        # masks: msk_lo [128, 8*512] (kt<8, all 512 queries);
        #        msk_hi [128, 8*256] (kt>=8, stripe-B queries only)
        mlo = np.zeros((128, 8 * NQ), np.float32)
        mhi = np.zeros((128, 8 * 256), np.float32)
        for kt in range(8):
            kpos = 128 * kt + np.arange(128)[:, None]
            mlo[:, 512 * kt:512 * (kt + 1)] = (
                (kpos <= qpos[None, :]) & pad[kpos])
        for kt in range(8, 16):
            kpos = 128 * kt + np.arange(128)[:, None]
            mhi[:, 256 * (kt - 8):256 * (kt - 7)] = (
                (kpos <= qpos[None, 256:]) & pad[kpos])
        in_maps.append({
            "wk_t": wk_t, "wv_t": wv_t, "wq_t": wq_t, "wo_t": wo_t,
            "xk_sl": np.ascontiguousarray(xk_T[b][:, 512 * j:512 * (j + 1)]),
            "xv_sl": np.ascontiguousarray(xv_T[b][:, 512 * j:512 * (j + 1)]),
            "xk_lo": np.ascontiguousarray(xk_T[b][:, 0:512]),
            "xv_lo": np.ascontiguousarray(xv_T[b][:, 0:512]),
            "xq_sl": xq,
            "bk_s": bk_s, "bq_s": bq_s, "bv_r": bv_r, "bo_r": bo_r,
            "ones1": ones1, "onesb": onesb,
            "msk_lo": mlo.astype(bf), "msk_hi": mhi.astype(bf),
        })

    _BUILT["last_maps"] = in_maps
    res = run_bass_kernel_spmd(nc, in_maps, core_ids=list(range(NCORES)))
    _BUILT["last_result"] = res

    outf = np.empty((B, S, D), np.float32)
    for core in range(NCORES):
        b, blkA, blkB = blocks[core]
        o = res.results[core]["out"]
        outf[b, 256 * blkA:256 * (blkA + 1)] = o[0:256]
        outf[b, 256 * blkB:256 * (blkB + 1)] = o[256:512]
    return outf


# revision 26
# speedup vs baseline: 1.0418x; 1.0418x over previous
"""Multi-headed causal attention (B=2, S=2048, D=1024, H=16, DK=DV=64) on 8
Trainium2 NeuronCores — v3.

Sharding: 2 groups of 4 cores, one group per batch element. Within a group,
core j owns two 256-query stripes (blocks j and 7-j, balanced causal work)
and computes the K/V projections for its own 512-key slice; slices are
AllGathered in bf16 inside the group. The gather is split into two
collectives by head-pair range (AG_A = K^T rows + V columns for pairs 0-3,
AG_B = pairs 4-7) so attention on pairs 0-3 can start while AG_B is still in
flight. The first collective call pays a ~80us CC-path init; to cover it,
every core also computes a local duplicate of the first 4 key-tiles and
attention is emitted in two program-order passes: pass 1 (key tiles 0-3,
local data only) runs during the gathers, spilling partial AV accumulators
and softmax denominators to SBUF; pass 2 restores them into PSUM with an
identity-matmul add and processes key tiles 4-15 pair-major in gather order.

All matmul inputs are bf16. Scores for the two heads of a pair run
concurrently in disjoint PE row groups (partitions 0:64 / 64:128). Softmax
skips max-subtraction; exp runs on ScalarE over [128,1024] PSUM tiles;
causal+padding masking is a bf16 multiplicative mask (host data, uniform
program), split between VectorE and GpSimd. The softmax denominator comes
from a 64-wide ones block in the AV stationary operand (AV-PSUM rows 0:64),
its reciprocal is a single-row custom-DVE approx-recip PE-replicated to 64
rows, and one VectorE multiply normalizes the per-pair attention output for
the PSUM-accumulated bf16 output projection.
"""

import numpy as np

B, S, D, H, DK = 2, 2048, 1024, 16, 64
NQ = 512          # queries per core: 2 stripes x 256
KSL = 512         # keys projected per core
NCORES = 8
NPAIR = 8         # head pairs
NKT = 16          # 128-key tiles

_BUILT = {}

# exp/mask tile groups: 4 "lo" groups of 2 key-tiles (free 512, both stripes)
# + 2 "hi" groups of 4 key-tiles (free 256, stripe B only)
GROUPS = [(0, (0, 1)), (1, (2, 3)), (2, (4, 5)), (3, (6, 7)),
          (4, (8, 9, 10, 11)), (5, (12, 13, 14, 15))]
PASS1 = GROUPS[:2]   # key tiles 0-3: computable from the local duplicate
PASS2 = GROUPS[2:]   # key tiles 4-15: need gathered data


def _build_nc():
    import os
    PH = int(os.environ.get("BISECT_PHASES", "9"))
    import concourse.bacc as bacc
    import concourse.mybir as mybir
    from concourse import tile
    from concourse.dve_ops import (
        RECIP_APPROX_FAST_CONSTS as RECIP_CONSTS,
        RECIPROCAL_APPROX_FAST as RECIP_FAST,
    )

    f32 = mybir.dt.float32
    f32r = mybir.dt.float32r
    bf16 = mybir.dt.bfloat16
    AF = mybir.ActivationFunctionType
    ALU = mybir.AluOpType

    nc = bacc.Bacc("TRN2", target_bir_lowering=False, debug=False,
                   num_devices=NCORES)

    wk_t = nc.declare_dram_parameter("wk_t", [D, D], bf16, isOutput=False)
    wv_t = nc.declare_dram_parameter("wv_t", [D, D], bf16, isOutput=False)
    wq_t = nc.declare_dram_parameter("wq_t", [D, D], bf16, isOutput=False)
    wo_t = nc.declare_dram_parameter("wo_t", [D, D], bf16, isOutput=False)
    xk_sl = nc.declare_dram_parameter("xk_sl", [D, KSL], bf16, isOutput=False)
    xv_sl = nc.declare_dram_parameter("xv_sl", [D, KSL], bf16, isOutput=False)
    xk_lo = nc.declare_dram_parameter("xk_lo", [D, KSL], bf16, isOutput=False)
    xv_lo = nc.declare_dram_parameter("xv_lo", [D, KSL], bf16, isOutput=False)
    xq_sl = nc.declare_dram_parameter("xq_sl", [D, NQ], bf16, isOutput=False)
    bk_s = nc.declare_dram_parameter("bk_s", [128, 8], f32, isOutput=False)
    bq_s = nc.declare_dram_parameter("bq_s", [128, 8], f32, isOutput=False)
    bv_r = nc.declare_dram_parameter("bv_r", [1, D], f32r, isOutput=False)
    bo_r = nc.declare_dram_parameter("bo_r", [1, D], f32r, isOutput=False)
    ones1 = nc.declare_dram_parameter("ones1", [1, 128], f32r, isOutput=False)
    onesb = nc.declare_dram_parameter("onesb", [128, D], bf16, isOutput=False)
    ident = nc.declare_dram_parameter("ident", [128, 128], bf16,
                                      isOutput=False)
    msk_lo = nc.declare_dram_parameter("msk_lo", [128, 8 * NQ], bf16,
                                       isOutput=False)
    msk_hi = nc.declare_dram_parameter("msk_hi", [128, 8 * 256], bf16,
                                       isOutput=False)
    out = nc.declare_dram_parameter("out", [NQ, D], f32, isOutput=True)

    RG = [[0, 1, 2, 3], [4, 5, 6, 7]]

    from contextlib import ExitStack

    class _Stop(Exception):
        pass

    with tile.TileContext(nc) as tc:
      try:
        with ExitStack() as ctx:
            persist = ctx.enter_context(tc.tile_pool(name="persist", bufs=1))
            dram = ctx.enter_context(
                tc.tile_pool(name="dram", bufs=1, space="DRAM"))

            # ---- constants ----
            bk_sb = persist.tile([128, 8], f32, name="bk")
            bq_sb = persist.tile([128, 8], f32, name="bq")
            ones_sb = persist.tile([1, 128], f32r, name="ones1")
            ident_sb = persist.tile([128, 128], bf16, name="ident")
            nc.sync.dma_start(bk_sb[:], bk_s[:])
            nc.sync.dma_start(bq_sb[:], bq_s[:])
            nc.sync.dma_start(ones_sb[:], ones1[:])
            nc.sync.dma_start(ident_sb[:], ident[:])
            msk_lo_sb = persist.tile([128, 8 * NQ], bf16, name="msklo")
            msk_hi_sb = persist.tile([128, 8 * 256], bf16, name="mskhi")
            nc.sync.dma_start(msk_lo_sb[:], msk_lo[:])
            nc.sync.dma_start(msk_hi_sb[:], msk_hi[:])

            # replicate bv across partitions (K=1 matmul)
            bv_rep = persist.tile([128, D], f32, name="bvrep")
            with tc.tile_pool(name="ps0", bufs=2, space="PSUM") as ps0, \
                 tc.tile_pool(name="p0s", bufs=1) as p0s:
                bv_rsb = p0s.tile([1, D], f32r, name="bvr")
                nc.sync.dma_start(bv_rsb[:], bv_r[:])
                for half in range(2):
                    rp = ps0.tile([128, 512], f32, name="rep0", tag="rep0")
                    nc.tensor.matmul(rp[:], ones_sb[:],
                                     bv_rsb[:, half * 512:(half + 1) * 512],
                                     start=True, stop=True)
                    nc.scalar.copy(bv_rep[:, half * 512:(half + 1) * 512],
                                   rp[:])

            # ---- AllGather DRAM bounce tiles (split by head-pair range) ----
            # per-rank payload: rows 0:512 = kT slice rows for 4 pairs,
            # rows 512:1024 = V slice (keys x 512 head dims)
            ag_in = [dram.tile([2 * KSL, KSL], bf16, name=f"agi{s}")
                     for s in range(2)]
            ag_out = [dram.tile([8 * KSL, KSL], bf16, name=f"ago{s}")
                      for s in range(2)]

            # ---- resident attention tensors ----
            kT_sb = [persist.tile([128, S], bf16, name=f"kt{hp}")
                     for hp in range(NPAIR)]
            v_sb = [persist.tile([128, 2048], bf16, name=f"v{kt}")
                    for kt in range(NKT)]
            qT_sb = [persist.tile([128, NQ], bf16, name=f"qt{hp}")
                     for hp in range(NPAIR)]
            navTn = [persist.tile([128, NQ], bf16, name=f"nv{hp}")
                     for hp in range(NPAIR)]
            # pass-1 partial AV accumulators (spilled from PSUM)
            acc = [persist.tile([128, NQ], bf16, name=f"acc{z}")
                   for z in range(2 * NPAIR)]

            # ---- P1: K/V slice projections + staged AllGathers ----
            with tc.tile_pool(name="wkx", bufs=1) as wkp, \
                 tc.tile_pool(name="slo", bufs=4) as slp, \
                 tc.tile_pool(name="psk", bufs=4, space="PSUM") as psk:
                wk_sb = [wkp.tile([128, D], bf16, name=f"wk{kp}",
                                  tag=f"wk{kp}") for kp in range(8)]
                xk_sb = [wkp.tile([128, KSL], bf16, name=f"xk{kp}",
                                  tag=f"xk{kp}") for kp in range(8)]
                wv_sb = [wkp.tile([128, D], bf16, name=f"wv{kp}",
                                  tag=f"wv{kp}") for kp in range(8)]
                xv_sb = [wkp.tile([128, KSL], bf16, name=f"xv{kp}",
                                  tag=f"xv{kp}") for kp in range(8)]
                for kp in range(8):
                    nc.sync.dma_start(wk_sb[kp][:],
                                      wk_t[kp * 128:(kp + 1) * 128, :])
                    nc.sync.dma_start(xk_sb[kp][:],
                                      xk_sl[kp * 128:(kp + 1) * 128, :])
                    nc.sync.dma_start(wv_sb[kp][:],
                                      wv_t[kp * 128:(kp + 1) * 128, :])
                    nc.sync.dma_start(xv_sb[kp][:],
                                      xv_sl[kp * 128:(kp + 1) * 128, :])

                def kslice(ft, dst_ap):
                    ps = psk.tile([128, KSL], f32, name="pk", tag="pk")
                    for kp in range(8):
                        nc.tensor.matmul(
                            ps[:], wk_sb[kp][:, ft * 128:(ft + 1) * 128],
                            xk_sb[kp][:], start=(kp == 0), stop=(kp == 7))
                    sl = slp.tile([128, KSL], bf16, name="ksl", tag="ksl")
                    nc.scalar.activation(sl[:], ps[:], AF.Identity,
                                         bias=bk_sb[:, ft:ft + 1])
                    nc.sync.dma_start(dst_ap, sl[:])

                def vslice(st, half, dst_ap):
                    ps = psk.tile([128, KSL], f32, name="pv", tag="pk")
                    for kp in range(8):
                        nc.tensor.matmul(
                            ps[:], xv_sb[kp][:, st * 128:(st + 1) * 128],
                            wv_sb[kp][:, half * 512:(half + 1) * 512],
                            start=(kp == 0), stop=(kp == 7))
                    sl = slp.tile([128, KSL], bf16, name="vsl", tag="ksl")
                    nc.vector.tensor_tensor(
                        sl[:], ps[:],
                        bv_rep[:, half * 512:(half + 1) * 512], ALU.add)
                    nc.sync.dma_start(dst_ap, sl[:])

                for stage in range(2):
                    for ft in range(4 * stage, 4 * stage + 4):
                        kslice(ft, ag_in[stage][(ft % 4) * 128:
                                                (ft % 4 + 1) * 128, :])
                    for st in range(4):
                        vslice(st, stage,
                               ag_in[stage][512 + st * 128:
                                            512 + (st + 1) * 128, :])
                    nc.gpsimd.collective_compute(
                        "AllGather", mybir.AluOpType.bypass,
                        replica_groups=RG, ins=[ag_in[stage][:].opt()],
                        outs=[ag_out[stage][:].opt()])

                if PH < 2:
                    raise _Stop()
                # ---- P2: local duplicate of keys 0:512 (kt 0-3) ----
                xkl_sb = [wkp.tile([128, KSL], bf16, name=f"xl{kp}",
                                   tag=f"xk{kp}") for kp in range(8)]
                xvl_sb = [wkp.tile([128, KSL], bf16, name=f"yl{kp}",
                                   tag=f"xv{kp}") for kp in range(8)]
                for kp in range(8):
                    nc.sync.dma_start(xkl_sb[kp][:],
                                      xk_lo[kp * 128:(kp + 1) * 128, :])
                    nc.sync.dma_start(xvl_sb[kp][:],
                                      xv_lo[kp * 128:(kp + 1) * 128, :])
                for ft in range(8):
                    ps = psk.tile([128, KSL], f32, name="pk", tag="pk")
                    for kp in range(8):
                        nc.tensor.matmul(
                            ps[:], wk_sb[kp][:, ft * 128:(ft + 1) * 128],
                            xkl_sb[kp][:], start=(kp == 0), stop=(kp == 7))
                    nc.scalar.activation(kT_sb[ft][:, 0:KSL], ps[:],
                                         AF.Identity,
                                         bias=bk_sb[:, ft:ft + 1])
                for st in range(4):
                    nc.sync.dma_start(
                        v_sb[st][:].rearrange("p (h c) -> p h c",
                                              c=128)[:, :, 0:64],
                        onesb[:].rearrange("p (h c) -> p h c", c=64))
                    for half in range(2):
                        ps = psk.tile([128, KSL], f32, name="pk", tag="pk")
                        for kp in range(8):
                            nc.tensor.matmul(
                                ps[:],
                                xvl_sb[kp][:, st * 128:(st + 1) * 128],
                                wv_sb[kp][:, half * 512:(half + 1) * 512],
                                start=(kp == 0), stop=(kp == 7))
                        nc.vector.tensor_tensor(
                            v_sb[st][:].rearrange(
                                "p (h c) -> p h c",
                                c=128)[:, half * 8:(half + 1) * 8, 64:128],
                            ps[:].rearrange("p (h c) -> p h c", c=64),
                            bv_rep[:, half * 512:(half + 1) * 512].rearrange(
                                "p (h c) -> p h c", c=64),
                            ALU.add)

            if PH < 3:
                raise _Stop()
            # ---- P3: Q projection (own 512 queries) ----
            with tc.tile_pool(name="wqx", bufs=1) as wqp, \
                 tc.tile_pool(name="psq", bufs=3, space="PSUM") as psq:
                wq_sb = [wqp.tile([128, D], bf16, name=f"wq{kp}",
                                  tag=f"w{kp}") for kp in range(8)]
                xq_sb = [wqp.tile([128, NQ], bf16, name=f"xq{kp}",
                                  tag=f"x{kp}") for kp in range(8)]
                for kp in range(8):
                    nc.sync.dma_start(wq_sb[kp][:],
                                      wq_t[kp * 128:(kp + 1) * 128, :])
                    nc.sync.dma_start(xq_sb[kp][:],
                                      xq_sl[kp * 128:(kp + 1) * 128, :])
                for ft in range(8):
                    ps = psq.tile([128, NQ], f32, name="pq", tag="pq")
                    for kp in range(8):
                        nc.tensor.matmul(
                            ps[:], wq_sb[kp][:, ft * 128:(ft + 1) * 128],
                            xq_sb[kp][:], start=(kp == 0), stop=(kp == 7))
                    nc.scalar.activation(qT_sb[ft][:], ps[:], AF.Identity,
                                         bias=bq_sb[:, ft:ft + 1])

            # prefetch Wo weights + bias (used in P7) before the gathered
            # loads so their DMAs aren't queued behind collective waits
            wop = ctx.enter_context(tc.tile_pool(name="wox", bufs=1))
            wo_sb = [wop.tile([128, D], bf16, name=f"wo{hp}")
                     for hp in range(NPAIR)]
            for hp in range(NPAIR):
                nc.sync.dma_start(wo_sb[hp][:],
                                  wo_t[hp * 128:(hp + 1) * 128, :])
            bo_rsb = wop.tile([1, D], f32r, name="bor")
            nc.sync.dma_start(bo_rsb[:], bo_r[:])

            if PH < 4:
                raise _Stop()

            attn_ctx = ExitStack()
            amp = attn_ctx.enter_context(tc.tile_pool(name="amp", bufs=6))
            recp = attn_ctx.enter_context(tc.tile_pool(name="recp", bufs=4))
            psc = attn_ctx.enter_context(
                tc.tile_pool(name="psc", bufs=2, space="PSUM"))
            psrep = attn_ctx.enter_context(
                tc.tile_pool(name="psrep", bufs=2, space="PSUM"))
            psav = attn_ctx.enter_context(
                tc.tile_pool(name="psav", bufs=2, space="PSUM"))

            def attn_groups(hp, groups, av):
                for g, kts in groups:
                    lo = g < 4
                    free = 512 if lo else 256
                    qoff = 0 if lo else 256
                    sc = [psc.tile([128, 1024], f32, name=f"sc{i}",
                                   tag="sc") for i in range(2)]
                    for ki, kt in enumerate(kts):
                        coff = ki * free
                        for i in range(2):
                            nc.tensor.matmul(
                                sc[i][:, coff:coff + free],
                                kT_sb[hp][64 * i:64 * i + 64,
                                          kt * 128:(kt + 1) * 128],
                                qT_sb[hp][64 * i:64 * i + 64,
                                          qoff:qoff + free],
                                start=True, stop=True)
                    msk = (msk_lo_sb[:, g * 1024:(g + 1) * 1024] if lo
                           else msk_hi_sb[:, (g - 4) * 1024:(g - 3) * 1024])
                    am = [amp.tile([128, 1024], bf16, name=f"am{i}",
                                   tag="am") for i in range(2)]
                    for i in range(2):
                        nc.scalar.activation(am[i][:], sc[i][:], AF.Exp,
                                             scale=0.125)
                        eng = nc.gpsimd if (not lo and hp % 2) else nc.vector
                        eng.tensor_tensor(am[i][:], am[i][:], msk, ALU.mult)
                    first, last = kts[0] == 0, kts[-1] == 15
                    for ki, kt in enumerate(kts):
                        coff = ki * free
                        for i in range(2):
                            nc.tensor.matmul(
                                av[i][:, qoff:qoff + free],
                                v_sb[kt][:, (2 * hp + i) * 128:
                                         (2 * hp + i + 1) * 128],
                                am[i][:, coff:coff + free],
                                start=(ki == 0 and first),
                                stop=(ki == len(kts) - 1 and last))

            # ---- P5: attention pass 1 — key tiles 0-3 from local data ----
            for hp in range(NPAIR):
                av = [psav.tile([128, NQ], f32, name=f"av{i}", tag="av")
                      for i in range(2)]
                attn_groups(hp, PASS1, av)
                for i in range(2):
                    nc.vector.tensor_copy(acc[2 * hp + i][:], av[i][:])

            if PH < 5:
                raise _Stop()
            # ---- P6: gathered loads + attention pass 2 (key tiles 4-15) ---
            for stage in range(2):
                for hp in range(4 * stage, 4 * stage + 4):
                    for c in range(1, 4):
                        nc.sync.dma_start(
                            kT_sb[hp][:, c * 512:(c + 1) * 512],
                            ag_out[stage][c * 1024 + (hp % 4) * 128:
                                          c * 1024 + (hp % 4 + 1) * 128, :])
                for kt in range(4, NKT):
                    c, st = kt // 4, kt % 4
                    nc.sync.dma_start(
                        v_sb[kt][:].rearrange(
                            "p (h c) -> p h c",
                            c=128)[:, 8 * stage:8 * stage + 8, 64:128],
                        ag_out[stage][c * 1024 + 512 + st * 128:
                                      c * 1024 + 512 + (st + 1) * 128,
                                      :].rearrange("p (h c) -> p h c", c=64))
            for kt in range(4, NKT):
                nc.sync.dma_start(
                    v_sb[kt][:].rearrange("p (h c) -> p h c", c=128)[:, :,
                                                                    0:64],
                    onesb[:].rearrange("p (h c) -> p h c", c=64))

            if PH < 6:
                raise _Stop()
            for hp in range(NPAIR):
                av = [psav.tile([128, NQ], f32, name=f"av{i}", tag="av")
                      for i in range(2)]
                # restore pass-1 partials into the fresh PSUM accumulators
                for i in range(2):
                    nc.tensor.matmul(av[i][:], ident_sb[:],
                                     acc[2 * hp + i][:], start=True,
                                     stop=False)
                attn_groups(hp, PASS2, av)
                # normalization: rows 0:64 of av hold 64 copies of the
                # softmax denominator (ones block in the AV stationary),
                # rows 64:128 the head's attention output
                for i in range(2):
                    rec = recp.tile([1, NQ], f32r, name="rec", tag="rec")
                    with nc.allow_low_precision(
                            reason="f32r recip feeds PE replicate"):
                        nc.vector._custom_dve(
                            RECIP_FAST, out=rec[:], in0=av[i][0:1, :],
                            s0=RECIP_CONSTS["s0"], s1=RECIP_CONSTS["s1"],
                            imm2=RECIP_CONSTS["imm2"])
                    rep = psrep.tile([64, NQ], f32, name="rep", tag="rep")
                    nc.tensor.matmul(rep[:], ones_sb[0:1, 0:64], rec[:],
                                     start=True, stop=True)
                    nc.vector.tensor_copy(
                        navTn[hp][64 * i:64 * i + 64, :],
                        av[i][64:128, :])
                    nc.vector.tensor_tensor(
                        navTn[hp][64 * i:64 * i + 64, :],
                        navTn[hp][64 * i:64 * i + 64, :],
                        rep[:], ALU.mult)

            attn_ctx.close()
            if PH < 7:
                raise _Stop()
            # ---- P7: output projection, all head pairs PSUM-accumulated ---
            with tc.tile_pool(name="fop", bufs=3) as fop, \
                 tc.tile_pool(name="bop", bufs=1) as bop, \
                 tc.tile_pool(name="pso", bufs=4, space="PSUM") as pso:
                bo_rep = bop.tile([128, D], f32, name="borep")
                for half in range(2):
                    rp = pso.tile([128, 512], f32, name="po", tag="po")
                    nc.tensor.matmul(rp[:], ones_sb[:],
                                     bo_rsb[:, half * 512:(half + 1) * 512],
                                     start=True, stop=True)
                    nc.scalar.copy(bo_rep[:, half * 512:(half + 1) * 512],
                                   rp[:])
                for oc in range(2):
                    for rc in range(4):
                        ps = pso.tile([128, 512], f32, name="po", tag="po")
                        for hp in range(NPAIR):
                            nc.tensor.matmul(
                                ps[:], navTn[hp][:, rc * 128:(rc + 1) * 128],
                                wo_sb[hp][:, oc * 512:(oc + 1) * 512],
                                start=(hp == 0), stop=(hp == 7))
                        fo = fop.tile([128, 512], f32, name="fo", tag="fo")
                        nc.vector.tensor_tensor(
                            fo[:], ps[:], bo_rep[:, oc * 512:(oc + 1) * 512],
                            ALU.add)
                        nc.sync.dma_start(
                            out[rc * 128:(rc + 1) * 128,
                                oc * 512:(oc + 1) * 512], fo[:])
      except _Stop:
          pass
    nc.compile()
    return nc


def kernel(V, K, Q, padding_mask, Wv_w, Wv_b, Wk_w, Wk_b, Wq_w, Wq_b,
           Wo_w, Wo_b):
    from concourse.bass_utils import run_bass_kernel_spmd
    import ml_dtypes

    bf = ml_dtypes.bfloat16
    V = np.asarray(V, np.float32)
    K = np.asarray(K, np.float32)
    Q = np.asarray(Q, np.float32)
    padding_mask = np.asarray(padding_mask)

    if "nc" not in _BUILT:
        _BUILT["nc"] = _build_nc()
    nc = _BUILT["nc"]

    wk_t = np.ascontiguousarray(np.asarray(Wk_w, np.float32).T).astype(bf)
    wv_t = np.ascontiguousarray(np.asarray(Wv_w, np.float32).T).astype(bf)
    wq_t = np.ascontiguousarray(np.asarray(Wq_w, np.float32).T).astype(bf)
    wo_t = np.ascontiguousarray(np.asarray(Wo_w, np.float32).T).astype(bf)
    bk_s = np.ascontiguousarray(np.asarray(Wk_b, np.float32).reshape(8, 128).T)
    bq_s = np.ascontiguousarray(np.asarray(Wq_b, np.float32).reshape(8, 128).T)
    bv_r = np.asarray(Wv_b, np.float32).reshape(1, D)
    bo_r = np.asarray(Wo_b, np.float32).reshape(1, D)
    ones1 = np.ones((1, 128), np.float32)
    onesb = np.ones((128, D), bf)
    ident = np.eye(128, dtype=np.float32).astype(bf)

    xk_T = [np.ascontiguousarray(K[b].T).astype(bf) for b in range(B)]
    xv_T = [np.ascontiguousarray(V[b].T).astype(bf) for b in range(B)]

    in_maps = []
    blocks = []
    for core in range(NCORES):
        b, j = core // 4, core % 4
        blkA, blkB = j, 7 - j
        blocks.append((b, blkA, blkB))
        rows = np.r_[256 * blkA:256 * (blkA + 1), 256 * blkB:256 * (blkB + 1)]
        xq = np.ascontiguousarray(Q[b][rows].T).astype(bf)
        qpos = np.r_[np.arange(256 * blkA, 256 * (blkA + 1)),
                     np.arange(256 * blkB, 256 * (blkB + 1))]
        pad = (padding_mask[b] != 0)
        # masks: msk_lo [128, 8*512] (kt<8, all 512 queries);
        #        msk_hi [128, 8*256] (kt>=8, stripe-B queries only)
        mlo = np.zeros((128, 8 * NQ), np.float32)
        mhi = np.zeros((128, 8 * 256), np.float32)
        for kt in range(8):
            kpos = 128 * kt + np.arange(128)[:, None]
            mlo[:, 512 * kt:512 * (kt + 1)] = (
                (kpos <= qpos[None, :]) & pad[kpos])
        for kt in range(8, 16):
            kpos = 128 * kt + np.arange(128)[:, None]
            mhi[:, 256 * (kt - 8):256 * (kt - 7)] = (
                (kpos <= qpos[None, 256:]) & pad[kpos])
        in_maps.append({
            "wk_t": wk_t, "wv_t": wv_t, "wq_t": wq_t, "wo_t": wo_t,
            "xk_sl": np.ascontiguousarray(xk_T[b][:, 512 * j:512 * (j + 1)]),
            "xv_sl": np.ascontiguousarray(xv_T[b][:, 512 * j:512 * (j + 1)]),
            "xk_lo": np.ascontiguousarray(xk_T[b][:, 0:512]),
            "xv_lo": np.ascontiguousarray(xv_T[b][:, 0:512]),
            "xq_sl": xq,
            "bk_s": bk_s, "bq_s": bq_s, "bv_r": bv_r, "bo_r": bo_r,
            "ones1": ones1, "onesb": onesb, "ident": ident,
            "msk_lo": mlo.astype(bf), "msk_hi": mhi.astype(bf),
        })

    _BUILT["last_maps"] = in_maps
    res = run_bass_kernel_spmd(nc, in_maps, core_ids=list(range(NCORES)))
    _BUILT["last_result"] = res

    outf = np.empty((B, S, D), np.float32)
    for core in range(NCORES):
        b, blkA, blkB = blocks[core]
        o = res.results[core]["out"]
        outf[b, 256 * blkA:256 * (blkA + 1)] = o[0:256]
        outf[b, 256 * blkB:256 * (blkB + 1)] = o[256:512]
    return outf
